# revision 2
# baseline (speedup 1.0000x reference)
"""Bidirectional cross-attention with talking heads — TRN2 Bass kernel, v2.

Sharding: 8 cores = 2 batches x 4 row-blocks of 256. Host-side, each core's
x/context are ROTATED so its query block sits at rows 0:256 — attention is
permutation-invariant over the key/value axis (softmax spans the full axis),
so key order doesn't matter and every core runs the same program with its
queries at a fixed offset. No per-core R-block inputs or projections needed:
the query-block projection is a slice of the full transposed projection.

Inputs are packed into 5 args (x, ctx, wpack[6], vecs[6], wexp[2]) and the
two outputs into one [512,1024] tensor: rows 0:256 = out, 256:512 = cout.
Arg binding through the tunnel costs ~0.1ms/arg, so fewer args matter.

Per-core schedule (all SBUF-resident, no DRAM spill):
  ctx side: LN -> PE-transpose (batched evictions) -> cqkT projection
            (feat on partitions) -> cv natural projection
  x side:   same -> qkT -> v
  path 1 (out):  K=cqkT, Q=qkT[:, :, 0:256], V=cv
  path 2 (cout): K=qkT,  Q=cqkT[:, :, 0:256], V=v

Per path, per head h:
  S^T[j,i] = KT_h^T @ Q_h   (PE, 64-deep contraction, PSUM [128,4,256] x2)
  U = exp(S^T * scale)      (ACT, one instr per 4 j-tiles)
  sigma = ones @ U          (PE accumulate over j tiles)
  U *= 1/sigma              (DVE, one 3D-broadcast instr per head)
  talking heads folded into A@V: for each h, out_h = U_h^T @ V(all heads),
  scaled per 64-col head chunk by W_th[g,h] and accumulated in SBUF (DVE).
  final: acc^T (PE transpose, batched evictions) @ W_out + b_out.

Masks are structurally all-ones for this problem; the reference's mask
application is the identity, so they are ignored. exp() without
max-subtraction is safe: |S*scale| <~ 7.
"""

import numpy as np
from contextlib import ExitStack

P = 128
N_TOK = 1024
DIM = 1024
HEADS = 16
DHEAD = 64
R = 256
SCALE = DHEAD ** -0.5
NCORES = 8

_CACHE = {}


def _patch_tile_drain(tile, mybir):
    """This container's walrus rejects >1 sync wait on an InstDrain
    ("Too many sync wait commands"). Split the TileContext tail drain's
    waits across a chain of single-wait drains on the same engine."""
    if getattr(tile.TileContext, "_drain_split_patched", False):
        return

    def _drain_and_barrier(self, tick_clock, wait_clock):
        drain_inst = self.nc.sync.drain()
        wait_clock.add_sem_waits(
            drain_inst.ins, tile.ScopedClock({None: tick_clock.global_clock})
        )
        si = drain_inst.ins.sync_info
        waits = list(si.on_wait) if si is not None else []
        if len(waits) > 1:
            drain_inst.ins.sync_info = mybir.SyncInfo(
                on_wait=[waits[0]], on_update=list(si.on_update)
            )
            for w in waits[1:]:
                extra = self.nc.sync.drain()
                extra.ins.sync_info = mybir.SyncInfo(on_wait=[w], on_update=[])

        self.nc.all_engine_barrier()
        assert self.sems is not None
        popped = self.nc._tile_sem_poison_stack.pop()
        assert popped is self._sem_poison
        self.nc.clear_and_free_semaphores(list(self.sems.allocated().values()))
        self.nc.all_engine_barrier()

    tile.TileContext._drain_and_barrier = _drain_and_barrier
    tile.TileContext._drain_split_patched = True


_WSPLIT_MAX = 1  # max sync waits this walrus accepts per instruction


def _patch_tile_wait_split(tile, mybir):
    """Split instructions carrying more than _WSPLIT_MAX sem-waits: move the
    excess onto same-engine NoOps committed immediately before (same basic
    block, so engine program order preserves the wait semantics)."""
    if getattr(tile.TileContext, "_wait_split_patched", False):
        return
    orig = tile.TileContext._commit_and_lower
    counter = [0]

    def _commit_and_lower(self, inst, *args, **kwargs):
        si = getattr(inst, "sync_info", None)
        eng = getattr(inst, "engine", None)
        if si is not None and eng is not None and len(si.on_wait) > _WSPLIT_MAX:
            waits = list(si.on_wait)
            keep = waits[-_WSPLIT_MAX:]
            for w in waits[:-_WSPLIT_MAX]:
                counter[0] += 1
                nop = mybir.InstNoOp(
                    name=f"I-wsplit-{counter[0]}",
                    engine=eng, ins=[], outs=[],
                    sync_info=mybir.SyncInfo(on_wait=[w], on_update=[]),
                )
                self._add_instruction(nop)
            inst.sync_info = mybir.SyncInfo(
                on_wait=keep, on_update=list(si.on_update)
            )
        return orig(self, inst, *args, **kwargs)

    tile.TileContext._commit_and_lower = _commit_and_lower
    tile.TileContext._wait_split_patched = True


# wpack / vecs slot order
W_QK, W_CQK, W_V, W_CV, W_OUT, W_COUT = range(6)
V_GX, V_BX, V_GC, V_BC, V_BOUT, V_BCOUT = range(6)


def build_program():
    import concourse.bass as bass
    import concourse.mybir as mybir
    import concourse.tile as tile
    from concourse.masks import make_identity

    _patch_tile_drain(tile, mybir)
    _patch_tile_wait_split(tile, mybir)

    f32 = mybir.dt.float32
    f32r = mybir.dt.float32r
    ts = bass.ts
    MULT = mybir.AluOpType.mult
    ADD = mybir.AluOpType.add
    AF = mybir.ActivationFunctionType

    nc = bass.Bass("TRN2", target_bir_lowering=False, debug=False)

    x_d = nc.dram_tensor("x", [N_TOK, DIM], f32, kind="ExternalInput")
    c_d = nc.dram_tensor("ctx", [N_TOK, DIM], f32, kind="ExternalInput")
    wpack_d = nc.dram_tensor("wpack", [6, DIM, DIM], f32, kind="ExternalInput")
    vecs_d = nc.dram_tensor("vecs", [6, P, DIM], f32, kind="ExternalInput")
    wexp_d = nc.dram_tensor("wexp", [2, P, HEADS, HEADS], f32,
                            kind="ExternalInput")
    res_d = nc.dram_tensor("res", [2 * R, DIM], f32, kind="ExternalOutput")

    def mm(out, lhsT, rhs, start, stop):
        nc.tensor.matmul(out, lhsT, rhs, start=start, stop=stop)

    with tile.TileContext(nc) as tc, ExitStack() as top:
        consts = top.enter_context(tc.tile_pool(name="consts", bufs=1))
        ident = consts.tile([P, P], f32)
        make_identity(nc, ident[:])
        onesM_f = consts.tile([P, P], f32)
        nc.vector.memset(onesM_f[:], 1.0)
        onesM = consts.tile([P, P], f32r)
        nc.vector.tensor_copy(onesM[:], onesM_f[:])
        eps = consts.tile([P, 1], f32)
        nc.vector.memset(eps[:], 1e-5)

        bigp = top.enter_context(tc.tile_pool(name="big", bufs=1))
        vp = top.enter_context(tc.tile_pool(name="vnat", bufs=1))

        def side_pipeline(ntp, src_d, gi, bi, wT_i, wV_i, v_dst):
            """LN+transpose src, project: returns dstT ([P,8,N_TOK] f32r,
            features on partitions); fills v_dst [P,8,DIM] (natural)."""
            nT = ntp.tile([P, 8, N_TOK], f32r, tag="nt", name="nT")

            with (
                tc.tile_pool(name="ln_consts", bufs=1) as lnp,
                tc.tile_pool(name="a_work", bufs=2) as aw,
                tc.tile_pool(name="a_small", bufs=4) as asm,
                tc.tile_pool(name="a_psum", bufs=2, space="PSUM") as aps,
            ):
                g_t = lnp.tile([P, DIM], f32, tag="g", name="g_t")
                nc.sync.dma_start(g_t[:], vecs_d[gi, :, :])
                b_t = lnp.tile([P, DIM], f32, tag="b", name="b_t")
                nc.sync.dma_start(b_t[:], vecs_d[bi, :, :])

                for it in range(8):
                    xt = aw.tile([P, DIM], f32, tag="t_a", name="xt")
                    nc.sync.dma_start(xt[:], src_d[ts(it, P), :])
                    nmean = asm.tile([P, 1], f32, tag="nmean", name="nmean")
                    nc.vector.reduce_sum(
                        nmean[:], xt[:], axis=mybir.AxisListType.X
                    )
                    nc.scalar.mul(nmean[:], nmean[:], -1.0 / DIM)
                    xc = aw.tile([P, DIM], f32, tag="t_b", name="xc")
                    nc.scalar.add(xc[:], xt[:], nmean[:])
                    sq = aw.tile([P, DIM], f32, tag="t_b", name="sq")
                    nc.scalar.activation(sq[:], xc[:], AF.Square)
                    var = asm.tile([P, 1], f32, tag="var", name="var")
                    nc.vector.reduce_sum(
                        var[:], sq[:], axis=mybir.AxisListType.X
                    )
                    nc.scalar.mul(var[:], var[:], 1.0 / DIM)
                    std = asm.tile([P, 1], f32, tag="std", name="std")
                    nc.scalar.activation(std[:], var[:], AF.Sqrt, bias=eps[:])
                    rstd = asm.tile([P, 1], f32, tag="rstd", name="rstd")
                    nc.vector.reciprocal(rstd[:], std[:])
                    xn = aw.tile([P, DIM], f32, tag="t_a", name="xn")
                    nc.vector.scalar_tensor_tensor(
                        xn[:], xc[:], rstd[:], g_t[:], op0=MULT, op1=MULT
                    )
                    nc.vector.tensor_add(xn[:], xn[:], b_t[:])
                    # transpose 1024 feats: 8x [128,128] PE passes, batched
                    # 4-per-PSUM-bank so evictions are [P,4,128] each
                    for fq in range(2):
                        pt = aps.tile([P, 4, P], f32, tag="tps", name="pt")
                        for fo in range(4):
                            nc.tensor.transpose(
                                pt[:, fo, :], xn[:, ts(4 * fq + fo, P)],
                                ident[:],
                            )
                        nc.any.tensor_copy(
                            nT[:, 4 * fq:4 * fq + 4, ts(it, P)], pt[:]
                        )

            dstT = bigp.tile([P, 8, N_TOK], f32r, tag=f"T{wT_i}",
                             name="dstT")

            with (
                tc.tile_pool(name="b_w", bufs=2) as bwp,
                tc.tile_pool(name="b_wpin", bufs=1) as bwpin,
                tc.tile_pool(name="b_psum", bufs=2, space="PSUM") as bps,
            ):
                # transposed projection: dstT[(hd), tok] = W^T @ nT
                for mt in range(8):
                    wcol = bwp.tile([P, 8, P], f32r, tag="wcol", name="wcol")
                    nc.sync.dma_start(
                        wcol[:],
                        wpack_d[wT_i, :, ts(mt, P)].bitcast(f32r)
                        .rearrange("(kt p) m -> p kt m", p=P),
                    )
                    pa = bps.tile([P, 512], f32, tag="pa", name="pa")
                    pb = bps.tile([P, 512], f32, tag="pb", name="pb")
                    for kt in range(8):
                        st, sp = kt == 0, kt == 7
                        mm(pa[:], wcol[:, kt, :], nT[:, kt, 0:512], st, sp)
                        mm(pb[:], wcol[:, kt, :], nT[:, kt, 512:1024], st, sp)
                    nc.any.tensor_copy(dstT[:, mt, 0:512], pa[:])
                    nc.any.tensor_copy(dstT[:, mt, 512:1024], pb[:])

                # natural projection: v[tok, (hd)] = nT^T @ W
                for nch in range(2):
                    wst = bwpin.tile([P, 8, 512], f32r, tag="wpin",
                                     name="wst")
                    nc.sync.dma_start(
                        wst[:],
                        wpack_d[wV_i, :, ts(nch, 512)].bitcast(f32r)
                        .rearrange("(kt p) m -> p kt m", p=P),
                    )
                    for mt in range(8):
                        ps = bps.tile([P, 512], f32, tag="pa", name="ps")
                        for kt in range(8):
                            mm(ps[:], nT[:, kt, ts(mt, P)], wst[:, kt, :],
                               kt == 0, kt == 7)
                        nc.any.tensor_copy(v_dst[:, mt, ts(nch, 512)], ps[:])
            return dstT

        with tc.tile_pool(name="nt", bufs=1) as ntp:
            cv = vp.tile([P, 8, DIM], f32r, tag="cv", name="cv")
            cqkT = side_pipeline(ntp, c_d, V_GC, V_BC, W_CQK, W_CV, cv)
            v = vp.tile([P, 8, DIM], f32r, tag="v", name="v")
            qkT = side_pipeline(ntp, x_d, V_GX, V_BX, W_QK, W_V, v)

        def run_path(KT, QT, V, wexp_i, wproj_i, bias_i, out_row0):
            """QT: transposed projection whose 0:256 token slice is the
            query block. Writes res rows [out_row0, out_row0+256)."""
            with (
                tc.tile_pool(name="p_small", bufs=1) as psm,
                tc.tile_pool(name="p_U", bufs=3) as pU,
                tc.tile_pool(name="p_sigs", bufs=2) as psigs,
                tc.tile_pool(name="p_tmp", bufs=3) as ptmp,
                tc.tile_pool(name="p_acc", bufs=1) as pacc,
                tc.tile_pool(name="p_scr", bufs=2, space="PSUM") as pscr,
                tc.tile_pool(name="p_sig", bufs=2, space="PSUM") as psig,
                tc.tile_pool(name="p_big", bufs=2, space="PSUM") as pbig,
            ):
                wexp = psm.tile([P, HEADS, HEADS], f32, name="wexp")
                nc.sync.dma_start(wexp[:], wexp_d[wexp_i, :, :, :])
                bias = psm.tile([P, DIM], f32, name="bias")
                nc.sync.dma_start(bias[:], vecs_d[bias_i, :, :])

                acc = pacc.tile([P, 2, DIM], f32, name="acc")

                for h in range(HEADS):
                    mt_h, off = h // 2, (h % 2) * DHEAD
                    U = pU.tile([P, 8, R], f32r, tag="U", name="U")
                    for half in range(2):
                        psS = pscr.tile([P, 4, R], f32, tag="scr", name="psS")
                        for j4 in range(4):
                            jt = 4 * half + j4
                            mm(
                                psS[:, j4, :],
                                KT[off:off + DHEAD, mt_h, ts(jt, P)],
                                QT[off:off + DHEAD, mt_h, 0:R],
                                True, True,
                            )
                        nc.scalar.activation(
                            U[:, 4 * half:4 * half + 4, :], psS[:],
                            AF.Exp, scale=SCALE,
                        )
                    sigps = psig.tile([P, R], f32, tag="sig", name="sigps")
                    for jt in range(8):
                        mm(sigps[:], onesM[:], U[:, jt, :], jt == 0, jt == 7)
                    siginv = psigs.tile([P, R], f32, tag="siginv",
                                        name="siginv")
                    nc.vector.reciprocal(siginv[:], sigps[:])
                    sbc = siginv[:, None, :].to_broadcast((P, 8, R))
                    nc.vector.tensor_mul(U[:, :, :], U[:, :, :], sbc)

                    # A@V with talking-heads mix folded in:
                    # po = U_h^T @ V (512-col chunks over all value heads g),
                    # scaled per 64-col chunk by W_th[g,h], accumulated.
                    for m2 in range(2):
                        for nch in range(2):
                            po = pbig.tile([P, 512], f32, tag="po", name="po")
                            for jt in range(8):
                                mm(po[:], U[:, jt, ts(m2, P)],
                                   V[:, jt, ts(nch, 512)], jt == 0, jt == 7)
                            wbc = wexp[:, h, ts(nch, 8)][:, :, None] \
                                .to_broadcast((P, 8, DHEAD))
                            po3 = po[:].rearrange("p (g d) -> p g d", d=DHEAD)
                            dst = acc[:, m2, ts(nch, 512)]
                            if h == 0:
                                nc.vector.tensor_mul(
                                    dst.rearrange("p (g d) -> p g d",
                                                  d=DHEAD),
                                    po3, wbc,
                                )
                            else:
                                tmp = ptmp.tile([P, 512], f32, tag="tmp",
                                                name="tmp")
                                nc.vector.tensor_mul(
                                    tmp[:].rearrange("p (g d) -> p g d",
                                                     d=DHEAD),
                                    po3, wbc,
                                )
                                nc.vector.tensor_add(dst, dst, tmp[:])

                # acc^T for the final contraction, batched 4-per-bank
                accT = pacc.tile([P, 8, R], f32r, name="accT")
                for m2 in range(2):
                    for gq in range(2):
                        pt = pscr.tile([P, 4, P], f32, tag="scr", name="ptf")
                        for go in range(4):
                            nc.tensor.transpose(
                                pt[:, go, :], acc[:, m2, ts(4 * gq + go, P)],
                                ident[:],
                            )
                        nc.any.tensor_copy(
                            accT[:, 4 * gq:4 * gq + 4, ts(m2, P)], pt[:]
                        )

                # final projection + bias
                with tc.tile_pool(name="p_wpin", bufs=1) as pwp:
                    for nch in range(2):
                        wst = pwp.tile([P, 8, 512], f32r, tag="wpin",
                                       name="wstf")
                        nc.sync.dma_start(
                            wst[:],
                            wpack_d[wproj_i, :, ts(nch, 512)].bitcast(f32r)
                            .rearrange("(kt p) m -> p kt m", p=P),
                        )
                        for m2 in range(2):
                            pf = pbig.tile([P, 512], f32, tag="po",
                                           name="pf")
                            for gdt in range(8):
                                mm(pf[:], accT[:, gdt, ts(m2, P)],
                                   wst[:, gdt, :], gdt == 0, gdt == 7)
                            ot = ptmp.tile([P, 512], f32, tag="ot",
                                           name="ot")
                            nc.vector.scalar_tensor_tensor(
                                ot[:], pf[:], 1.0, bias[:, ts(nch, 512)],
                                op0=MULT, op1=ADD,
                            )
                            nc.sync.dma_start(
                                res_d[out_row0 + m2 * P:
                                      out_row0 + (m2 + 1) * P,
                                      ts(nch, 512)],
                                ot[:],
                            )

        # path 1: out rows (x queries attend to context; values = cv)
        run_path(cqkT, qkT, cv, 0, W_OUT, V_BOUT, 0)
        # path 2: cout rows (context queries attend to x; values = v)
        run_path(qkT, cqkT, v, 1, W_COUT, V_BCOUT, R)

    return nc


def _prep_in_maps(inputs):
    g = lambda k: np.ascontiguousarray(np.asarray(inputs[k], dtype=np.float32))
    x = g("x")
    ctx = g("context")
    wpack = np.stack([g("W_qk"), g("W_cqk"), g("W_v"), g("W_cv"),
                      g("W_out"), g("W_cout")])
    bcast = lambda v: np.broadcast_to(
        np.asarray(v, np.float32), (P, DIM)
    )
    vecs = np.ascontiguousarray(np.stack([
        bcast(inputs["ln_g"]), bcast(inputs["ln_b"]),
        bcast(inputs["cln_g"]), bcast(inputs["cln_b"]),
        bcast(inputs["b_out"]), bcast(inputs["b_cout"]),
    ]))
    wexp = np.ascontiguousarray(np.stack([
        np.broadcast_to(g("W_th").T[None, :, :], (P, HEADS, HEADS)),
        np.broadcast_to(g("W_cth").T[None, :, :], (P, HEADS, HEADS)),
    ]))
    in_maps = []
    for c in range(NCORES):
        b, r0 = c // 4, (c % 4) * R
        # rotate so this core's query block is rows 0:R (key/value order
        # under full-axis softmax is permutation-invariant)
        xr = np.ascontiguousarray(np.roll(x[b], -r0, axis=0))
        cr = np.ascontiguousarray(np.roll(ctx[b], -r0, axis=0))
        in_maps.append({
            "x": xr, "ctx": cr,
            "wpack": wpack, "vecs": vecs, "wexp": wexp,
        })
    return in_maps


def kernel(**inputs):
    from concourse.bass_utils import run_bass_kernel_spmd

    if "nc" not in _CACHE:
        _CACHE["nc"] = build_program()
    nc = _CACHE["nc"]

    in_maps = _prep_in_maps(inputs)
    res = run_bass_kernel_spmd(nc, in_maps, core_ids=list(range(NCORES)))

    out = np.empty((2, N_TOK, DIM), np.float32)
    cout = np.empty((2, N_TOK, DIM), np.float32)
    for c in range(NCORES):
        b, r0 = c // 4, (c % 4) * R
        blk = res.results[c]["res"]
        out[b, r0:r0 + R] = blk[0:R]
        cout[b, r0:r0 + R] = blk[R:2 * R]
    return out, cout


# revision 6
# speedup vs baseline: 1.0360x; 1.0360x over previous
"""Bidirectional cross-attention with talking heads — TRN2 Bass kernel, v2.

Sharding: 8 cores = 2 batches x 4 row-blocks of 256. Host-side, each core's
x/context are ROTATED so its query block sits at rows 0:256 — attention is
permutation-invariant over the key/value axis (softmax spans the full axis),
so key order doesn't matter and every core runs the same program with its
queries at a fixed offset. No per-core R-block inputs or projections needed:
the query-block projection is a slice of the full transposed projection.

Inputs are packed into 5 args (x, ctx, wpack[6], vecs[6], wexp[2]) and the
two outputs into one [512,1024] tensor: rows 0:256 = out, 256:512 = cout.
Arg binding through the tunnel costs ~0.1ms/arg, so fewer args matter.

Per-core schedule (all SBUF-resident, no DRAM spill):
  ctx side: LN -> PE-transpose (batched evictions) -> cqkT projection
            (feat on partitions) -> cv natural projection
  x side:   same -> qkT -> v
  path 1 (out):  K=cqkT, Q=qkT[:, :, 0:256], V=cv
  path 2 (cout): K=qkT,  Q=cqkT[:, :, 0:256], V=v

Per path, per head h:
  S^T[j,i] = KT_h^T @ Q_h   (PE, 64-deep contraction, PSUM [128,4,256] x2)
  U = exp(S^T * scale)      (ACT, one instr per 4 j-tiles)
  sigma = ones @ U          (PE accumulate over j tiles)
  U *= 1/sigma              (DVE, one 3D-broadcast instr per head)
  talking heads folded into A@V: for each h, out_h = U_h^T @ V(all heads),
  scaled per 64-col head chunk by W_th[g,h] and accumulated in SBUF (DVE).
  final: acc^T (PE transpose, batched evictions) @ W_out + b_out.

Masks are structurally all-ones for this problem; the reference's mask
application is the identity, so they are ignored. exp() without
max-subtraction is safe: |S*scale| <~ 7.
"""

import numpy as np
from contextlib import ExitStack

P = 128
N_TOK = 1024
DIM = 1024
HEADS = 16
DHEAD = 64
R = 256
SCALE = DHEAD ** -0.5
NCORES = 8

_CACHE = {}


def _patch_tile_drain(tile, mybir):
    """This container's walrus rejects >1 sync wait on an InstDrain
    ("Too many sync wait commands"). Split the TileContext tail drain's
    waits across a chain of single-wait drains on the same engine."""
    if getattr(tile.TileContext, "_drain_split_patched", False):
        return

    def _drain_and_barrier(self, tick_clock, wait_clock):
        drain_inst = self.nc.sync.drain()
        wait_clock.add_sem_waits(
            drain_inst.ins, tile.ScopedClock({None: tick_clock.global_clock})
        )
        si = drain_inst.ins.sync_info
        waits = list(si.on_wait) if si is not None else []
        if len(waits) > 1:
            drain_inst.ins.sync_info = mybir.SyncInfo(
                on_wait=[waits[0]], on_update=list(si.on_update)
            )
            for w in waits[1:]:
                extra = self.nc.sync.drain()
                extra.ins.sync_info = mybir.SyncInfo(on_wait=[w], on_update=[])

        self.nc.all_engine_barrier()
        assert self.sems is not None
        popped = self.nc._tile_sem_poison_stack.pop()
        assert popped is self._sem_poison
        self.nc.clear_and_free_semaphores(list(self.sems.allocated().values()))
        self.nc.all_engine_barrier()

    tile.TileContext._drain_and_barrier = _drain_and_barrier
    tile.TileContext._drain_split_patched = True


_WSPLIT_MAX = 1  # max sync waits this walrus accepts per instruction


def _patch_tile_wait_split(tile, mybir):
    """Split instructions carrying more than _WSPLIT_MAX sem-waits: move the
    excess onto same-engine NoOps committed immediately before (same basic
    block, so engine program order preserves the wait semantics)."""
    if getattr(tile.TileContext, "_wait_split_patched", False):
        return
    orig = tile.TileContext._commit_and_lower
    counter = [0]

    def _commit_and_lower(self, inst, *args, **kwargs):
        si = getattr(inst, "sync_info", None)
        eng = getattr(inst, "engine", None)
        if si is not None and eng is not None and len(si.on_wait) > _WSPLIT_MAX:
            waits = list(si.on_wait)
            keep = waits[-_WSPLIT_MAX:]
            for w in waits[:-_WSPLIT_MAX]:
                counter[0] += 1
                nop = mybir.InstNoOp(
                    name=f"I-wsplit-{counter[0]}",
                    engine=eng, ins=[], outs=[],
                    sync_info=mybir.SyncInfo(on_wait=[w], on_update=[]),
                )
                self._add_instruction(nop)
            inst.sync_info = mybir.SyncInfo(
                on_wait=keep, on_update=list(si.on_update)
            )
        return orig(self, inst, *args, **kwargs)

    tile.TileContext._commit_and_lower = _commit_and_lower
    tile.TileContext._wait_split_patched = True


# wpack / vecs slot order
W_QK, W_CQK, W_V, W_CV, W_OUT, W_COUT = range(6)
V_GX, V_BX, V_GC, V_BC, V_BOUT, V_BCOUT = range(6)


def build_program():
    import concourse.bass as bass
    import concourse.mybir as mybir
    import concourse.tile as tile
    from concourse.masks import make_identity

    _patch_tile_drain(tile, mybir)
    _patch_tile_wait_split(tile, mybir)

    f32 = mybir.dt.float32
    f32r = mybir.dt.float32r
    ts = bass.ts
    MULT = mybir.AluOpType.mult
    ADD = mybir.AluOpType.add
    AF = mybir.ActivationFunctionType

    nc = bass.Bass("TRN2", target_bir_lowering=False, debug=False)

    x_d = nc.dram_tensor("x", [N_TOK, DIM], f32, kind="ExternalInput")
    c_d = nc.dram_tensor("ctx", [N_TOK, DIM], f32, kind="ExternalInput")
    wpack_d = nc.dram_tensor("wpack", [6, DIM, DIM], f32, kind="ExternalInput")
    vecs_d = nc.dram_tensor("vecs", [6, P, DIM], f32, kind="ExternalInput")
    wexp_d = nc.dram_tensor("wexp", [2, P, HEADS, HEADS], f32,
                            kind="ExternalInput")
    res_d = nc.dram_tensor("res", [2 * R, DIM], f32, kind="ExternalOutput")

    def mm(out, lhsT, rhs, start, stop):
        nc.tensor.matmul(out, lhsT, rhs, start=start, stop=stop)

    with tile.TileContext(nc) as tc, ExitStack() as top:
        consts = top.enter_context(tc.tile_pool(name="consts", bufs=1))
        ident = consts.tile([P, P], f32)
        make_identity(nc, ident[:])
        onesM_f = consts.tile([P, P], f32)
        nc.vector.memset(onesM_f[:], 1.0)
        onesM = consts.tile([P, P], f32r)
        nc.vector.tensor_copy(onesM[:], onesM_f[:])
        eps = consts.tile([P, 1], f32)
        nc.vector.memset(eps[:], 1e-5)

        bigp = top.enter_context(tc.tile_pool(name="big", bufs=1))
        vp = top.enter_context(tc.tile_pool(name="vnat", bufs=1))

        def side_pipeline(ntp, src_d, gi, bi, wT_i, wV_i, v_dst):
            """LN+transpose src, project: returns dstT ([P,8,N_TOK] f32r,
            features on partitions); fills v_dst [P,8,DIM] (natural)."""
            nT = ntp.tile([P, 8, N_TOK], f32r, tag="nt", name="nT")

            with (
                tc.tile_pool(name="ln_consts", bufs=1) as lnp,
                tc.tile_pool(name="a_work", bufs=2) as aw,
                tc.tile_pool(name="a_small", bufs=4) as asm,
                tc.tile_pool(name="a_psum", bufs=2, space="PSUM") as aps,
            ):
                g_t = lnp.tile([P, DIM], f32, tag="g", name="g_t")
                nc.sync.dma_start(g_t[:], vecs_d[gi, :, :])
                b_t = lnp.tile([P, DIM], f32, tag="b", name="b_t")
                nc.sync.dma_start(b_t[:], vecs_d[bi, :, :])

                for it in range(8):
                    xt = aw.tile([P, DIM], f32, tag="t_a", name="xt")
                    nc.sync.dma_start(xt[:], src_d[ts(it, P), :])
                    nmean = asm.tile([P, 1], f32, tag="nmean", name="nmean")
                    nc.vector.reduce_sum(
                        nmean[:], xt[:], axis=mybir.AxisListType.X
                    )
                    nc.scalar.mul(nmean[:], nmean[:], -1.0 / DIM)
                    xc = aw.tile([P, DIM], f32, tag="t_b", name="xc")
                    nc.scalar.add(xc[:], xt[:], nmean[:])
                    sq = aw.tile([P, DIM], f32, tag="t_b", name="sq")
                    nc.scalar.activation(sq[:], xc[:], AF.Square)
                    var = asm.tile([P, 1], f32, tag="var", name="var")
                    nc.vector.reduce_sum(
                        var[:], sq[:], axis=mybir.AxisListType.X
                    )
                    nc.scalar.mul(var[:], var[:], 1.0 / DIM)
                    std = asm.tile([P, 1], f32, tag="std", name="std")
                    nc.scalar.activation(std[:], var[:], AF.Sqrt, bias=eps[:])
                    rstd = asm.tile([P, 1], f32, tag="rstd", name="rstd")
                    nc.vector.reciprocal(rstd[:], std[:])
                    xn = aw.tile([P, DIM], f32, tag="t_a", name="xn")
                    nc.vector.scalar_tensor_tensor(
                        xn[:], xc[:], rstd[:], g_t[:], op0=MULT, op1=MULT
                    )
                    nc.vector.tensor_add(xn[:], xn[:], b_t[:])
                    # transpose 1024 feats: 8x [128,128] PE passes, batched
                    # 4-per-PSUM-bank so evictions are [P,4,128] each
                    for fq in range(2):
                        pt = aps.tile([P, 4, P], f32, tag="tps", name="pt")
                        for fo in range(4):
                            nc.tensor.transpose(
                                pt[:, fo, :], xn[:, ts(4 * fq + fo, P)],
                                ident[:],
                            )
                        nc.any.tensor_copy(
                            nT[:, 4 * fq:4 * fq + 4, ts(it, P)], pt[:]
                        )

            dstT = bigp.tile([P, 8, N_TOK], f32r, tag=f"T{wT_i}",
                             name="dstT")

            with (
                tc.tile_pool(name="b_w", bufs=2) as bwp,
                tc.tile_pool(name="b_wpin", bufs=1) as bwpin,
                tc.tile_pool(name="b_psum", bufs=2, space="PSUM") as bps,
            ):
                # transposed projection: dstT[(hd), tok] = W^T @ nT
                for mt in range(8):
                    wcol = bwp.tile([P, 8, P], f32r, tag="wcol", name="wcol")
                    nc.sync.dma_start(
                        wcol[:],
                        wpack_d[wT_i, :, ts(mt, P)].bitcast(f32r)
                        .rearrange("(kt p) m -> p kt m", p=P),
                    )
                    pa = bps.tile([P, 512], f32, tag="pa", name="pa")
                    pb = bps.tile([P, 512], f32, tag="pb", name="pb")
                    for kt in range(8):
                        st, sp = kt == 0, kt == 7
                        mm(pa[:], wcol[:, kt, :], nT[:, kt, 0:512], st, sp)
                        mm(pb[:], wcol[:, kt, :], nT[:, kt, 512:1024], st, sp)
                    nc.any.tensor_copy(dstT[:, mt, 0:512], pa[:])
                    nc.any.tensor_copy(dstT[:, mt, 512:1024], pb[:])

                # natural projection: v[tok, (hd)] = nT^T @ W
                for nch in range(2):
                    wst = bwpin.tile([P, 8, 512], f32r, tag="wpin",
                                     name="wst")
                    nc.sync.dma_start(
                        wst[:],
                        wpack_d[wV_i, :, ts(nch, 512)].bitcast(f32r)
                        .rearrange("(kt p) m -> p kt m", p=P),
                    )
                    for mt in range(8):
                        ps = bps.tile([P, 512], f32, tag="pa", name="ps")
                        for kt in range(8):
                            mm(ps[:], nT[:, kt, ts(mt, P)], wst[:, kt, :],
                               kt == 0, kt == 7)
                        nc.any.tensor_copy(v_dst[:, mt, ts(nch, 512)], ps[:])
            return dstT

        with tc.tile_pool(name="nt", bufs=1) as ntp:
            cv = vp.tile([P, 8, DIM], f32r, tag="cv", name="cv")
            cqkT = side_pipeline(ntp, c_d, V_GC, V_BC, W_CQK, W_CV, cv)
            v = vp.tile([P, 8, DIM], f32r, tag="v", name="v")
            qkT = side_pipeline(ntp, x_d, V_GX, V_BX, W_QK, W_V, v)

        def run_path(KT, QT, V, wexp_i, wproj_i, bias_i, out_row0):
            """QT: transposed projection whose 0:256 token slice is the
            query block. Writes res rows [out_row0, out_row0+256)."""
            with (
                tc.tile_pool(name="p_small", bufs=1) as psm,
                tc.tile_pool(name="p_U", bufs=3) as pU,
                tc.tile_pool(name="p_sigs", bufs=2) as psigs,
                tc.tile_pool(name="p_tmp", bufs=3) as ptmp,
                tc.tile_pool(name="p_acc", bufs=1) as pacc,
                tc.tile_pool(name="p_scr", bufs=2, space="PSUM") as pscr,
                tc.tile_pool(name="p_sig", bufs=2, space="PSUM") as psig,
                tc.tile_pool(name="p_big", bufs=2, space="PSUM") as pbig,
            ):
                wexp = psm.tile([P, HEADS, HEADS], f32, name="wexp")
                nc.sync.dma_start(wexp[:], wexp_d[wexp_i, :, :, :])
                bias = psm.tile([P, DIM], f32, name="bias")
                nc.sync.dma_start(bias[:], vecs_d[bias_i, :, :])

                acc = pacc.tile([P, 2, DIM], f32, name="acc")

                for h in range(HEADS):
                    mt_h, off = h // 2, (h % 2) * DHEAD
                    U = pU.tile([P, 8, R], f32r, tag="U", name="U")
                    for half in range(2):
                        psS = pscr.tile([P, 4, R], f32, tag="scr", name="psS")
                        for j4 in range(4):
                            jt = 4 * half + j4
                            mm(
                                psS[:, j4, :],
                                KT[off:off + DHEAD, mt_h, ts(jt, P)],
                                QT[off:off + DHEAD, mt_h, 0:R],
                                True, True,
                            )
                        nc.scalar.activation(
                            U[:, 4 * half:4 * half + 4, :], psS[:],
                            AF.Exp, scale=SCALE,
                        )
                    sigps = psig.tile([P, R], f32, tag="sig", name="sigps")
                    for jt in range(8):
                        mm(sigps[:], onesM[:], U[:, jt, :], jt == 0, jt == 7)
                    siginv = psigs.tile([P, R], f32, tag="siginv",
                                        name="siginv")
                    nc.vector.reciprocal(siginv[:], sigps[:])
                    sbc = siginv[:, None, :].to_broadcast((P, 8, R))
                    nc.vector.tensor_mul(U[:, :, :], U[:, :, :], sbc)

                    # A@V with talking-heads mix folded in:
                    # po = U_h^T @ V (512-col chunks over all value heads g),
                    # scaled per 64-col chunk by W_th[g,h], accumulated.
                    for m2 in range(2):
                        for nch in range(2):
                            po = pbig.tile([P, 512], f32, tag="po", name="po")
                            for jt in range(8):
                                mm(po[:], U[:, jt, ts(m2, P)],
                                   V[:, jt, ts(nch, 512)], jt == 0, jt == 7)
                            wbc = wexp[:, h, ts(nch, 8)][:, :, None] \
                                .to_broadcast((P, 8, DHEAD))
                            po3 = po[:].rearrange("p (g d) -> p g d", d=DHEAD)
                            dst = acc[:, m2, ts(nch, 512)]
                            if h == 0:
                                nc.vector.tensor_mul(
                                    dst.rearrange("p (g d) -> p g d",
                                                  d=DHEAD),
                                    po3, wbc,
                                )
                            else:
                                tmp = ptmp.tile([P, 512], f32, tag="tmp",
                                                name="tmp")
                                nc.vector.tensor_mul(
                                    tmp[:].rearrange("p (g d) -> p g d",
                                                     d=DHEAD),
                                    po3, wbc,
                                )
                                nc.vector.tensor_add(dst, dst, tmp[:])

                # acc^T for the final contraction, batched 4-per-bank
                accT = pacc.tile([P, 8, R], f32r, name="accT")
                for m2 in range(2):
                    for gq in range(2):
                        pt = pscr.tile([P, 4, P], f32, tag="scr", name="ptf")
                        for go in range(4):
                            nc.tensor.transpose(
                                pt[:, go, :], acc[:, m2, ts(4 * gq + go, P)],
                                ident[:],
                            )
                        nc.any.tensor_copy(
                            accT[:, 4 * gq:4 * gq + 4, ts(m2, P)], pt[:]
                        )

                # final projection + bias
                with tc.tile_pool(name="p_wpin", bufs=1) as pwp:
                    for nch in range(2):
                        wst = pwp.tile([P, 8, 512], f32r, tag="wpin",
                                       name="wstf")
                        nc.sync.dma_start(
                            wst[:],
                            wpack_d[wproj_i, :, ts(nch, 512)].bitcast(f32r)
                            .rearrange("(kt p) m -> p kt m", p=P),
                        )
                        for m2 in range(2):
                            pf = pbig.tile([P, 512], f32, tag="po",
                                           name="pf")
                            for gdt in range(8):
                                mm(pf[:], accT[:, gdt, ts(m2, P)],
                                   wst[:, gdt, :], gdt == 0, gdt == 7)
                            ot = ptmp.tile([P, 512], f32, tag="ot",
                                           name="ot")
                            nc.vector.scalar_tensor_tensor(
                                ot[:], pf[:], 1.0, bias[:, ts(nch, 512)],
                                op0=MULT, op1=ADD,
                            )
                            nc.sync.dma_start(
                                res_d[out_row0 + m2 * P:
                                      out_row0 + (m2 + 1) * P,
                                      ts(nch, 512)],
                                ot[:],
                            )

        # path 1: out rows (x queries attend to context; values = cv)
        run_path(cqkT, qkT, cv, 0, W_OUT, V_BOUT, 0)
        # path 2: cout rows (context queries attend to x; values = v)
        run_path(qkT, cqkT, v, 1, W_COUT, V_BCOUT, R)

    return nc


def _prep_in_maps(inputs):
    g = lambda k: np.ascontiguousarray(np.asarray(inputs[k], dtype=np.float32))
    x = g("x")
    ctx = g("context")
    wpack = np.stack([g("W_qk"), g("W_cqk"), g("W_v"), g("W_cv"),
                      g("W_out"), g("W_cout")])
    bcast = lambda v: np.broadcast_to(
        np.asarray(v, np.float32), (P, DIM)
    )
    vecs = np.ascontiguousarray(np.stack([
        bcast(inputs["ln_g"]), bcast(inputs["ln_b"]),
        bcast(inputs["cln_g"]), bcast(inputs["cln_b"]),
        bcast(inputs["b_out"]), bcast(inputs["b_cout"]),
    ]))
    wexp = np.ascontiguousarray(np.stack([
        np.broadcast_to(g("W_th").T[None, :, :], (P, HEADS, HEADS)),
        np.broadcast_to(g("W_cth").T[None, :, :], (P, HEADS, HEADS)),
    ]))
    in_maps = []
    for c in range(NCORES):
        b, r0 = c // 4, (c % 4) * R
        # rotate so this core's query block is rows 0:R (key/value order
        # under full-axis softmax is permutation-invariant)
        xr = np.ascontiguousarray(np.roll(x[b], -r0, axis=0))
        cr = np.ascontiguousarray(np.roll(ctx[b], -r0, axis=0))
        in_maps.append({
            "x": xr, "ctx": cr,
            "wpack": wpack, "vecs": vecs, "wexp": wexp,
        })
    return in_maps


def kernel(**inputs):
    from concourse.bass_utils import run_bass_kernel_spmd

    if "nc" not in _CACHE:
        _CACHE["nc"] = build_program()
    nc = _CACHE["nc"]

    in_maps = _prep_in_maps(inputs)
    res = run_bass_kernel_spmd(nc, in_maps, core_ids=list(range(NCORES)))

    out = np.empty((2, N_TOK, DIM), np.float32)
    cout = np.empty((2, N_TOK, DIM), np.float32)
    for c in range(NCORES):
        b, r0 = c // 4, (c % 4) * R
        blk = res.results[c]["res"]
        out[b, r0:r0 + R] = blk[0:R]
        cout[b, r0:r0 + R] = blk[R:2 * R]
    return out, cout


# revision 8
# speedup vs baseline: 1.1985x; 1.1568x over previous
"""Bidirectional cross-attention with talking heads — TRN2 Bass kernel, v2.

Sharding: 8 cores = 2 batches x 4 row-blocks of 256. Host-side, each core's
x/context are ROTATED so its query block sits at rows 0:256 — attention is
permutation-invariant over the key/value axis (softmax spans the full axis),
so key order doesn't matter and every core runs the same program with its
queries at a fixed offset. No per-core R-block inputs or projections needed:
the query-block projection is a slice of the full transposed projection.

Inputs are packed into 5 args (x, ctx, wpack[6], vecs[6], wexp[2]) and the
two outputs into one [512,1024] tensor: rows 0:256 = out, 256:512 = cout.
Arg binding through the tunnel costs ~0.1ms/arg, so fewer args matter.

Per-core schedule (all SBUF-resident, no DRAM spill):
  ctx side: LN -> PE-transpose (batched evictions) -> cqkT projection
            (feat on partitions) -> cv natural projection
  x side:   same -> qkT -> v
  path 1 (out):  K=cqkT, Q=qkT[:, :, 0:256], V=cv
  path 2 (cout): K=qkT,  Q=cqkT[:, :, 0:256], V=v

Per path, per head h:
  S^T[j,i] = KT_h^T @ Q_h   (PE, 64-deep contraction, PSUM [128,4,256] x2)
  U = exp(S^T * scale)      (ACT, one instr per 4 j-tiles)
  sigma = ones @ U          (PE accumulate over j tiles)
  U *= 1/sigma              (DVE, one 3D-broadcast instr per head)
  talking heads folded into A@V: for each h, out_h = U_h^T @ V(all heads),
  scaled per 64-col head chunk by W_th[g,h] and accumulated in SBUF (DVE).
  final: acc^T (PE transpose, batched evictions) @ W_out + b_out.

Masks are structurally all-ones for this problem; the reference's mask
application is the identity, so they are ignored. exp() without
max-subtraction is safe: |S*scale| <~ 7.
"""

import numpy as np
from contextlib import ExitStack

P = 128
N_TOK = 1024
DIM = 1024
HEADS = 16
DHEAD = 64
R = 256
SCALE = DHEAD ** -0.5
NCORES = 8

_CACHE = {}


def _patch_tile_drain(tile, mybir):
    """This container's walrus rejects >1 sync wait on an InstDrain
    ("Too many sync wait commands"). Split the TileContext tail drain's
    waits across a chain of single-wait drains on the same engine."""
    if getattr(tile.TileContext, "_drain_split_patched", False):
        return

    def _drain_and_barrier(self, tick_clock, wait_clock):
        drain_inst = self.nc.sync.drain()
        wait_clock.add_sem_waits(
            drain_inst.ins, tile.ScopedClock({None: tick_clock.global_clock})
        )
        si = drain_inst.ins.sync_info
        waits = list(si.on_wait) if si is not None else []
        if len(waits) > 1:
            drain_inst.ins.sync_info = mybir.SyncInfo(
                on_wait=[waits[0]], on_update=list(si.on_update)
            )
            for w in waits[1:]:
                extra = self.nc.sync.drain()
                extra.ins.sync_info = mybir.SyncInfo(on_wait=[w], on_update=[])

        self.nc.all_engine_barrier()
        assert self.sems is not None
        popped = self.nc._tile_sem_poison_stack.pop()
        assert popped is self._sem_poison
        self.nc.clear_and_free_semaphores(list(self.sems.allocated().values()))
        self.nc.all_engine_barrier()

    tile.TileContext._drain_and_barrier = _drain_and_barrier
    tile.TileContext._drain_split_patched = True


_WSPLIT_MAX = 1  # max sync waits this walrus accepts per instruction


def _patch_tile_wait_split(tile, mybir):
    """Split instructions carrying more than _WSPLIT_MAX sem-waits: move the
    excess onto same-engine NoOps committed immediately before (same basic
    block, so engine program order preserves the wait semantics)."""
    if getattr(tile.TileContext, "_wait_split_patched", False):
        return
    orig = tile.TileContext._commit_and_lower
    counter = [0]

    def _commit_and_lower(self, inst, *args, **kwargs):
        si = getattr(inst, "sync_info", None)
        eng = getattr(inst, "engine", None)
        if si is not None and eng is not None and len(si.on_wait) > _WSPLIT_MAX:
            waits = list(si.on_wait)
            keep = waits[-_WSPLIT_MAX:]
            for w in waits[:-_WSPLIT_MAX]:
                counter[0] += 1
                nop = mybir.InstNoOp(
                    name=f"I-wsplit-{counter[0]}",
                    engine=eng, ins=[], outs=[],
                    sync_info=mybir.SyncInfo(on_wait=[w], on_update=[]),
                )
                self._add_instruction(nop)
            inst.sync_info = mybir.SyncInfo(
                on_wait=keep, on_update=list(si.on_update)
            )
        return orig(self, inst, *args, **kwargs)

    tile.TileContext._commit_and_lower = _commit_and_lower
    tile.TileContext._wait_split_patched = True


# wpack / vecs slot order
W_QK, W_CQK, W_V, W_CV, W_OUT, W_COUT = range(6)
V_GX, V_BX, V_GC, V_BC, V_BOUT, V_BCOUT = range(6)


def build_program():
    import concourse.bass as bass
    import concourse.mybir as mybir
    import concourse.tile as tile
    from concourse.masks import make_identity

    _patch_tile_drain(tile, mybir)
    _patch_tile_wait_split(tile, mybir)

    f32 = mybir.dt.float32
    f32r = mybir.dt.float32r
    ts = bass.ts
    MULT = mybir.AluOpType.mult
    ADD = mybir.AluOpType.add
    AF = mybir.ActivationFunctionType

    nc = bass.Bass("TRN2", target_bir_lowering=False, debug=False)

    x_d = nc.dram_tensor("x", [N_TOK, DIM], f32, kind="ExternalInput")
    c_d = nc.dram_tensor("ctx", [N_TOK, DIM], f32, kind="ExternalInput")
    wpack_d = nc.dram_tensor("wpack", [6, DIM, DIM], f32, kind="ExternalInput")
    vecs_d = nc.dram_tensor("vecs", [6, P, DIM], f32, kind="ExternalInput")
    wexp_d = nc.dram_tensor("wexp", [2, P, HEADS, HEADS], f32,
                            kind="ExternalInput")
    res_d = nc.dram_tensor("res", [2 * R, DIM], f32, kind="ExternalOutput")

    def mm(out, lhsT, rhs, start, stop):
        nc.tensor.matmul(out, lhsT, rhs, start=start, stop=stop)

    with tile.TileContext(nc) as tc, ExitStack() as top:
        consts = top.enter_context(tc.tile_pool(name="consts", bufs=1))
        ident = consts.tile([P, P], f32)
        make_identity(nc, ident[:])
        onesM_f = consts.tile([P, P], f32)
        nc.vector.memset(onesM_f[:], 1.0)
        onesM = consts.tile([P, P], f32r)
        nc.vector.tensor_copy(onesM[:], onesM_f[:])
        eps = consts.tile([P, 1], f32)
        nc.vector.memset(eps[:], 1e-5)

        bigp = top.enter_context(tc.tile_pool(name="big", bufs=1))
        vp = top.enter_context(tc.tile_pool(name="vnat", bufs=1))

        def side_pipeline(ntp, src_d, gi, bi, wT_i, wV_i, v_dst):
            """LN+transpose src, project: returns dstT ([P,8,N_TOK] f32r,
            features on partitions); fills v_dst [P,8,DIM] (natural)."""
            nT = ntp.tile([P, 8, N_TOK], f32r, tag="nt", name="nT")

            with (
                tc.tile_pool(name="ln_consts", bufs=1) as lnp,
                tc.tile_pool(name="a_work", bufs=2) as aw,
                tc.tile_pool(name="a_small", bufs=4) as asm,
                tc.tile_pool(name="a_psum", bufs=2, space="PSUM") as aps,
            ):
                g_t = lnp.tile([P, DIM], f32, tag="g", name="g_t")
                nc.sync.dma_start(g_t[:], vecs_d[gi, :, :])
                b_t = lnp.tile([P, DIM], f32, tag="b", name="b_t")
                nc.sync.dma_start(b_t[:], vecs_d[bi, :, :])

                for it in range(8):
                    xt = aw.tile([P, DIM], f32, tag="t_a", name="xt")
                    nc.sync.dma_start(xt[:], src_d[ts(it, P), :])
                    # var = E[x^2] - mu^2: the Square reads xt directly so
                    # ACT and DVE reduces run concurrently (shorter chain
                    # than center-then-square)
                    s1 = asm.tile([P, 1], f32, tag="s1", name="s1")
                    nc.vector.reduce_sum(
                        s1[:], xt[:], axis=mybir.AxisListType.X
                    )
                    sq = aw.tile([P, DIM], f32, tag="t_b", name="sq")
                    nc.scalar.activation(sq[:], xt[:], AF.Square)
                    s2 = asm.tile([P, 1], f32, tag="s2", name="s2")
                    nc.vector.reduce_sum(
                        s2[:], sq[:], axis=mybir.AxisListType.X
                    )
                    nmean = asm.tile([P, 1], f32, tag="nmean", name="nmean")
                    nc.scalar.mul(nmean[:], s1[:], -1.0 / DIM)
                    e2 = asm.tile([P, 1], f32, tag="e2", name="e2")
                    nc.scalar.mul(e2[:], s2[:], 1.0 / DIM)
                    musq = asm.tile([P, 1], f32, tag="musq", name="musq")
                    nc.vector.tensor_mul(musq[:], nmean[:], nmean[:])
                    var = asm.tile([P, 1], f32, tag="var", name="var")
                    nc.vector.tensor_tensor(
                        var[:], e2[:], musq[:], op=mybir.AluOpType.subtract
                    )
                    std = asm.tile([P, 1], f32, tag="std", name="std")
                    nc.scalar.activation(std[:], var[:], AF.Sqrt, bias=eps[:])
                    rstd = asm.tile([P, 1], f32, tag="rstd", name="rstd")
                    nc.vector.reciprocal(rstd[:], std[:])
                    xc = aw.tile([P, DIM], f32, tag="t_b", name="xc")
                    nc.scalar.add(xc[:], xt[:], nmean[:])
                    xn = aw.tile([P, DIM], f32, tag="t_a", name="xn")
                    nc.vector.scalar_tensor_tensor(
                        xn[:], xc[:], rstd[:], g_t[:], op0=MULT, op1=MULT
                    )
                    nc.vector.tensor_add(xn[:], xn[:], b_t[:])
                    # transpose 1024 feats: 8x [128,128] PE passes, batched
                    # 4-per-PSUM-bank so evictions are [P,4,128] each
                    for fq in range(2):
                        pt = aps.tile([P, 4, P], f32, tag="tps", name="pt")
                        for fo in range(4):
                            nc.tensor.transpose(
                                pt[:, fo, :], xn[:, ts(4 * fq + fo, P)],
                                ident[:],
                            )
                        nc.any.tensor_copy(
                            nT[:, 4 * fq:4 * fq + 4, ts(it, P)], pt[:]
                        )

            dstT = bigp.tile([P, 8, N_TOK], f32r, tag=f"T{wT_i}",
                             name="dstT")

            with (
                tc.tile_pool(name="b_w", bufs=2) as bwp,
                tc.tile_pool(name="b_wpin", bufs=1) as bwpin,
                tc.tile_pool(name="b_psum", bufs=2, space="PSUM") as bps,
            ):
                # transposed projection: dstT[(hd), tok] = W^T @ nT
                for mt in range(8):
                    wcol = bwp.tile([P, 8, P], f32r, tag="wcol", name="wcol")
                    nc.sync.dma_start(
                        wcol[:],
                        wpack_d[wT_i, :, ts(mt, P)].bitcast(f32r)
                        .rearrange("(kt p) m -> p kt m", p=P),
                    )
                    pa = bps.tile([P, 512], f32, tag="pa", name="pa")
                    pb = bps.tile([P, 512], f32, tag="pb", name="pb")
                    for kt in range(8):
                        st, sp = kt == 0, kt == 7
                        mm(pa[:], wcol[:, kt, :], nT[:, kt, 0:512], st, sp)
                        mm(pb[:], wcol[:, kt, :], nT[:, kt, 512:1024], st, sp)
                    nc.any.tensor_copy(dstT[:, mt, 0:512], pa[:])
                    nc.any.tensor_copy(dstT[:, mt, 512:1024], pb[:])

                # natural projection: v[tok, (hd)] = nT^T @ W
                for nch in range(2):
                    wst = bwpin.tile([P, 8, 512], f32r, tag="wpin",
                                     name="wst")
                    nc.sync.dma_start(
                        wst[:],
                        wpack_d[wV_i, :, ts(nch, 512)].bitcast(f32r)
                        .rearrange("(kt p) m -> p kt m", p=P),
                    )
                    for mt in range(8):
                        ps = bps.tile([P, 512], f32, tag="pa", name="ps")
                        for kt in range(8):
                            mm(ps[:], nT[:, kt, ts(mt, P)], wst[:, kt, :],
                               kt == 0, kt == 7)
                        nc.any.tensor_copy(v_dst[:, mt, ts(nch, 512)], ps[:])
            return dstT

        with tc.tile_pool(name="nt", bufs=1) as ntp:
            cv = vp.tile([P, 8, DIM], f32r, tag="cv", name="cv")
            cqkT = side_pipeline(ntp, c_d, V_GC, V_BC, W_CQK, W_CV, cv)
            v = vp.tile([P, 8, DIM], f32r, tag="v", name="v")
            qkT = side_pipeline(ntp, x_d, V_GX, V_BX, W_QK, W_V, v)

        def run_path(KT, QT, V, wexp_i, wproj_i, bias_i, out_row0):
            """QT: transposed projection whose 0:256 token slice is the
            query block. Writes res rows [out_row0, out_row0+256)."""
            with (
                tc.tile_pool(name="p_small", bufs=1) as psm,
                tc.tile_pool(name="p_U", bufs=3) as pU,
                tc.tile_pool(name="p_sigs", bufs=2) as psigs,
                tc.tile_pool(name="p_tmp", bufs=3) as ptmp,
                tc.tile_pool(name="p_acc", bufs=1) as pacc,
                tc.tile_pool(name="p_scr", bufs=2, space="PSUM") as pscr,
                tc.tile_pool(name="p_sig", bufs=2, space="PSUM") as psig,
                tc.tile_pool(name="p_big", bufs=2, space="PSUM") as pbig,
            ):
                wexp = psm.tile([P, HEADS, HEADS], f32, name="wexp")
                nc.sync.dma_start(wexp[:], wexp_d[wexp_i, :, :, :])
                bias = psm.tile([P, DIM], f32, name="bias")
                nc.sync.dma_start(bias[:], vecs_d[bias_i, :, :])

                acc = pacc.tile([P, 2, DIM], f32, name="acc")

                for h in range(HEADS):
                    mt_h, off = h // 2, (h % 2) * DHEAD
                    U = pU.tile([P, 8, R], f32r, tag="U", name="U")
                    for half in range(2):
                        psS = pscr.tile([P, 4, R], f32, tag="scr", name="psS")
                        for j4 in range(4):
                            jt = 4 * half + j4
                            mm(
                                psS[:, j4, :],
                                KT[off:off + DHEAD, mt_h, ts(jt, P)],
                                QT[off:off + DHEAD, mt_h, 0:R],
                                True, True,
                            )
                        nc.scalar.activation(
                            U[:, 4 * half:4 * half + 4, :], psS[:],
                            AF.Exp, scale=SCALE,
                        )
                    sigps = psig.tile([P, R], f32, tag="sig", name="sigps")
                    for jt in range(8):
                        mm(sigps[:], onesM[:], U[:, jt, :], jt == 0, jt == 7)
                    siginv = psigs.tile([P, R], f32, tag="siginv",
                                        name="siginv")
                    nc.vector.reciprocal(siginv[:], sigps[:])
                    sbc = siginv[:, None, :].to_broadcast((P, 8, R))
                    nc.vector.tensor_mul(U[:, :, :], U[:, :, :], sbc)

                    # A@V with talking-heads mix folded in:
                    # po = U_h^T @ V (512-col chunks over all value heads g),
                    # scaled per 64-col chunk by W_th[g,h], accumulated.
                    for m2 in range(2):
                        for nch in range(2):
                            po = pbig.tile([P, 512], f32, tag="po", name="po")
                            for jt in range(8):
                                mm(po[:], U[:, jt, ts(m2, P)],
                                   V[:, jt, ts(nch, 512)], jt == 0, jt == 7)
                            wbc = wexp[:, h, ts(nch, 8)][:, :, None] \
                                .to_broadcast((P, 8, DHEAD))
                            po3 = po[:].rearrange("p (g d) -> p g d", d=DHEAD)
                            dst = acc[:, m2, ts(nch, 512)]
                            if h == 0:
                                nc.vector.tensor_mul(
                                    dst.rearrange("p (g d) -> p g d",
                                                  d=DHEAD),
                                    po3, wbc,
                                )
                            else:
                                tmp = ptmp.tile([P, 512], f32, tag="tmp",
                                                name="tmp")
                                nc.vector.tensor_mul(
                                    tmp[:].rearrange("p (g d) -> p g d",
                                                     d=DHEAD),
                                    po3, wbc,
                                )
                                nc.vector.tensor_add(dst, dst, tmp[:])

                # acc^T for the final contraction, batched 4-per-bank
                accT = pacc.tile([P, 8, R], f32r, name="accT")
                for m2 in range(2):
                    for gq in range(2):
                        pt = pscr.tile([P, 4, P], f32, tag="scr", name="ptf")
                        for go in range(4):
                            nc.tensor.transpose(
                                pt[:, go, :], acc[:, m2, ts(4 * gq + go, P)],
                                ident[:],
                            )
                        nc.any.tensor_copy(
                            accT[:, 4 * gq:4 * gq + 4, ts(m2, P)], pt[:]
                        )

                # final projection + bias
                with tc.tile_pool(name="p_wpin", bufs=1) as pwp:
                    for nch in range(2):
                        wst = pwp.tile([P, 8, 512], f32r, tag="wpin",
                                       name="wstf")
                        nc.sync.dma_start(
                            wst[:],
                            wpack_d[wproj_i, :, ts(nch, 512)].bitcast(f32r)
                            .rearrange("(kt p) m -> p kt m", p=P),
                        )
                        for m2 in range(2):
                            pf = pbig.tile([P, 512], f32, tag="po",
                                           name="pf")
                            for gdt in range(8):
                                mm(pf[:], accT[:, gdt, ts(m2, P)],
                                   wst[:, gdt, :], gdt == 0, gdt == 7)
                            ot = ptmp.tile([P, 512], f32, tag="ot",
                                           name="ot")
                            nc.vector.scalar_tensor_tensor(
                                ot[:], pf[:], 1.0, bias[:, ts(nch, 512)],
                                op0=MULT, op1=ADD,
                            )
                            nc.sync.dma_start(
                                res_d[out_row0 + m2 * P:
                                      out_row0 + (m2 + 1) * P,
                                      ts(nch, 512)],
                                ot[:],
                            )

        # path 1: out rows (x queries attend to context; values = cv)
        run_path(cqkT, qkT, cv, 0, W_OUT, V_BOUT, 0)
        # path 2: cout rows (context queries attend to x; values = v)
        run_path(qkT, cqkT, v, 1, W_COUT, V_BCOUT, R)

    return nc


def _prep_in_maps(inputs):
    g = lambda k: np.ascontiguousarray(np.asarray(inputs[k], dtype=np.float32))
    x = g("x")
    ctx = g("context")
    wpack = np.stack([g("W_qk"), g("W_cqk"), g("W_v"), g("W_cv"),
                      g("W_out"), g("W_cout")])
    bcast = lambda v: np.broadcast_to(
        np.asarray(v, np.float32), (P, DIM)
    )
    vecs = np.ascontiguousarray(np.stack([
        bcast(inputs["ln_g"]), bcast(inputs["ln_b"]),
        bcast(inputs["cln_g"]), bcast(inputs["cln_b"]),
        bcast(inputs["b_out"]), bcast(inputs["b_cout"]),
    ]))
    wexp = np.ascontiguousarray(np.stack([
        np.broadcast_to(g("W_th").T[None, :, :], (P, HEADS, HEADS)),
        np.broadcast_to(g("W_cth").T[None, :, :], (P, HEADS, HEADS)),
    ]))
    in_maps = []
    for c in range(NCORES):
        b, r0 = c // 4, (c % 4) * R
        # rotate so this core's query block is rows 0:R (key/value order
        # under full-axis softmax is permutation-invariant)
        xr = np.ascontiguousarray(np.roll(x[b], -r0, axis=0))
        cr = np.ascontiguousarray(np.roll(ctx[b], -r0, axis=0))
        in_maps.append({
            "x": xr, "ctx": cr,
            "wpack": wpack, "vecs": vecs, "wexp": wexp,
        })
    return in_maps


def kernel(**inputs):
    from concourse.bass_utils import run_bass_kernel_spmd

    if "nc" not in _CACHE:
        _CACHE["nc"] = build_program()
    nc = _CACHE["nc"]

    in_maps = _prep_in_maps(inputs)
    res = run_bass_kernel_spmd(nc, in_maps, core_ids=list(range(NCORES)))

    out = np.empty((2, N_TOK, DIM), np.float32)
    cout = np.empty((2, N_TOK, DIM), np.float32)
    for c in range(NCORES):
        b, r0 = c // 4, (c % 4) * R
        blk = res.results[c]["res"]
        out[b, r0:r0 + R] = blk[0:R]
        cout[b, r0:r0 + R] = blk[R:2 * R]
    return out, cout
